# revision 30
# baseline (speedup 1.0000x reference)
"""Trainium2 Bass kernel for nn_DenseFusionLoss (DenseFusion pose-estimation loss).

Strategy: data-parallel over the batch axis. Each of the 8 NeuronCores gets 4
batches (poses/confidences/class_ids shard) plus the full replicated
[21,2048,3] vertex table. Each core computes partial sums
[sum_selected_add_loss, sum_softplus, sum_pose_reg]; the host combines the 8
partial vectors into the final scalar loss.

Device-side computation per core (all heavy math on-device):
  - quat -> rotation matrices via unnormalized-product form (scaled by 1/|q|^2)
  - vertex gather fused into the point-transform matmul: lhsT[(c,i), d] =
    onehot[c] * R[d,i] (K=63), rhs = vertex table laid out [63, 2048]
  - pairwise squared distances d2[v,w] = pn[v] + gn[w] - 2 p.g via a K=5
    matmul with lhsT rows [-2px,-2py,-2pz, pn, 1], rhs rows [gx,gy,gz, 1, gn]
  - ADD-S: DVE reduce_min over PSUM d2 tiles, clamp, sqrt, mean
  - ADD: true-difference form on GPSIMD + PE partition-sum + ACT sqrt-accum
  - conf loss: softplus(-x) = Ln(1 + Exp(-x)) with ACT accumulate
  - pose reg: relu(|t|-2)^2 via ACT
"""

from contextlib import ExitStack

import numpy as np

import concourse.bass as bass
import concourse.bacc as bacc
import concourse.tile as tile
from concourse import mybir
from concourse.bass_utils import run_bass_kernel_spmd

B, C, V, NCONF = 32, 21, 2048, 1024
NCORES = 8
BPC = B // NCORES  # batches per core
F32 = mybir.dt.float32
I32 = mybir.dt.int32
AF = mybir.ActivationFunctionType
OP = mybir.AluOpType
AX = mybir.AxisListType

ADD_WEIGHT = 1.0
CONF_WEIGHT = 0.1
POSE_REG_WEIGHT = 0.1

_CACHE = {}


def _emit(nc, tc, h, ctx):
    pool = {}
    pool["setup"] = ctx.enter_context(tc.tile_pool(name="setup", bufs=1))
    pool["acc"] = ctx.enter_context(tc.tile_pool(name="acc", bufs=1))
    pool["ab"] = ctx.enter_context(tc.tile_pool(name="ab", bufs=2))
    pool["work"] = ctx.enter_context(tc.tile_pool(name="work", bufs=2))
    pool["psB"] = ctx.enter_context(tc.tile_pool(name="psB", bufs=3, space="PSUM"))
    pool["psS"] = ctx.enter_context(tc.tile_pool(name="psS", bufs=2, space="PSUM"))
    pool["dram"] = ctx.enter_context(tc.tile_pool(name="dram", bufs=1, space="DRAM"))

    setup = pool["setup"]
    acc = pool["acc"]
    work = pool["work"]
    psS = pool["psS"]
    psB = pool["psB"]

    # ---------------- constant / input loads ----------------
    iota21 = setup.tile([21, 1], F32, tag="iota21")
    nc.sync.dma_start(out=iota21, in_=h["iota21"].ap())
    ones = setup.tile([128, 1], F32, tag="ones")
    nc.sync.dma_start(out=ones, in_=h["ones"].ap())

    # vertex table, natural contiguous layout [c, (v i)]; the pts matmul
    # reads strided [21, 512] coordinate-plane views out of it
    vnat = setup.tile([C, V * 3], F32, tag="vnat")
    nc.sync.dma_start(out=vnat, in_=h["verts"].ap())
    vview = vnat[:].rearrange("c (v i) -> c v i", i=3)

    poses = setup.tile([8, 7], F32, tag="poses")
    nc.sync.dma_start(out=poses, in_=h["poses"].ap())

    # t4[d, j] = poses[j, d] for d in 0..3 (row 3 is junk, masked to 0 below)
    t4 = setup.tile([4, 8], F32, tag="t4")
    for j in range(8):
        nc.sync.dma_start(
            out=t4[:, j : j + 1],
            in_=bass.AP(tensor=h["poses"].ap().tensor, offset=j * 7, ap=[[1, 4]]),
        )
    mask_a = setup.tile([4, 1], F32, tag="mask_a")
    nc.sync.dma_start(out=mask_a, in_=h["mask_a"].ap())
    mask_g = setup.tile([4, 1], F32, tag="mask_g")
    nc.sync.dma_start(out=mask_g, in_=h["mask_g"].ap())
    e3x3 = setup.tile([3, 4], F32, tag="e3x3")
    nc.sync.dma_start(out=e3x3, in_=h["e3x3"].ap())
    e1x4 = setup.tile([1, 4], F32, tag="e1x4")
    nc.sync.dma_start(out=e1x4, in_=h["e1x4"].ap())
    ones512 = setup.tile([1, 512], F32, tag="ones512")
    nc.sync.dma_start(out=ones512, in_=h["ones512"].ap())
    # per-pose ACT bias columns: rows 0-2 = (-2|1) * t, row 3 = 0
    bias_a = setup.tile([4, 8], F32, tag="bias_a")
    nc.vector.tensor_scalar(
        out=bias_a, in0=t4, scalar1=mask_a, scalar2=None, op0=OP.mult
    )
    bias_g = setup.tile([4, 8], F32, tag="bias_g")
    nc.vector.tensor_scalar(
        out=bias_g, in0=t4, scalar1=mask_g, scalar2=None, op0=OP.mult
    )

    conf = setup.tile([BPC, NCONF], F32, tag="conf")
    nc.sync.dma_start(out=conf, in_=h["conf"].ap())

    # ---------------- quaternion -> rotation matrices ----------------
    q = poses[:, 3:7]
    qsq = setup.tile([8, 4], F32, tag="qsq")
    nc.vector.tensor_mul(qsq, q, q)
    nrm2 = setup.tile([8, 1], F32, tag="nrm2")
    nc.vector.tensor_reduce(out=nrm2, in_=qsq, axis=AX.X, op=OP.add)
    inv2 = setup.tile([8, 1], F32, tag="inv2")
    nc.vector.reciprocal(inv2, nrm2)
    s2 = setup.tile([8, 1], F32, tag="s2")
    nc.vector.tensor_scalar_mul(s2, inv2, 2.0)
    ns2 = setup.tile([8, 1], F32, tag="ns2")
    nc.vector.tensor_scalar_mul(ns2, inv2, -2.0)

    # cross products: xy xz yz wx wy wz
    pr = setup.tile([8, 6], F32, tag="pr")
    nc.vector.tensor_mul(pr[:, 0:1], q[:, 1:2], q[:, 2:3])  # xy
    nc.vector.tensor_mul(pr[:, 1:2], q[:, 1:2], q[:, 3:4])  # xz
    nc.vector.tensor_mul(pr[:, 2:3], q[:, 2:3], q[:, 3:4])  # yz
    nc.vector.tensor_mul(pr[:, 3:4], q[:, 0:1], q[:, 1:2])  # wx
    nc.vector.tensor_mul(pr[:, 4:5], q[:, 0:1], q[:, 2:3])  # wy
    nc.vector.tensor_mul(pr[:, 5:6], q[:, 0:1], q[:, 3:4])  # wz

    xx, yy, zz = qsq[:, 1:2], qsq[:, 2:3], qsq[:, 3:4]
    xy, xz, yz = pr[:, 0:1], pr[:, 1:2], pr[:, 2:3]
    wx, wy, wz = pr[:, 3:4], pr[:, 4:5], pr[:, 5:6]

    sm = setup.tile([8, 9], F32, tag="sm")
    # entry e: (a op b); diag entries get ns2*sum + 1, off-diag s2*sum
    entries = [
        (yy, zz, OP.add, True),   # R00 = 1 - 2(yy+zz)/n2
        (xy, wz, OP.subtract, False),  # R01
        (xz, wy, OP.add, False),  # R02
        (xy, wz, OP.add, False),  # R10
        (xx, zz, OP.add, True),   # R11
        (yz, wx, OP.subtract, False),  # R12
        (xz, wy, OP.subtract, False),  # R20
        (yz, wx, OP.add, False),  # R21
        (xx, yy, OP.add, True),   # R22
    ]
    r_all = setup.tile([8, 9], F32, tag="r_all")
    for e, (a, b_, op, diag) in enumerate(entries):
        nc.vector.tensor_tensor(out=sm[:, e : e + 1], in0=a, in1=b_, op=op)
        nc.vector.tensor_scalar(
            out=r_all[:, e : e + 1],
            in0=sm[:, e : e + 1],
            scalar1=ns2 if diag else s2,
            scalar2=1.0 if diag else 0.0,
            op0=OP.mult,
            op1=OP.add,
        )

    # ---------------- one-hot class rows ----------------
    cls21 = setup.tile([21, 4], I32, tag="cls21")
    nc.gpsimd.dma_start(
        out=cls21,
        in_=bass.AP(tensor=h["cls"].ap().tensor, offset=0, ap=[[0, 21], [1, 4]]),
    )
    cls21f = setup.tile([21, 4], F32, tag="cls21f")
    nc.vector.tensor_copy(out=cls21f, in_=cls21)
    oh21 = setup.tile([21, 4], F32, tag="oh21")
    nc.vector.tensor_scalar(
        out=oh21, in0=cls21f, scalar1=iota21, scalar2=None, op0=OP.is_equal
    )

    # ---------------- replicated transform lhsT ----------------
    # l72[c, col], col = d*24 + side*12 + b*3 + i  holds
    # onehot_b(c) * R_side,b[d, i]  (pred side scaled by -2)
    r_dram = pool["dram"].tile([8, 9], F32, tag="r_dram")
    nc.sync.dma_start(out=r_dram, in_=r_all)
    rt72 = setup.tile([C, 72], F32, tag="rt72")
    for d in range(3):
        for side in range(2):
            # dst cols (b, i) at fixed (d, side); src r_dram[j, d*3+i]
            nc.gpsimd.dma_start(
                out=bass.AP(
                    tensor=rt72.tensor,
                    offset=rt72.offset + d * 24 + side * 12,
                    ap=[rt72.ap[0], [3, 4], [1, 3]],
                ),
                in_=bass.AP(
                    tensor=r_dram.tensor,
                    offset=r_dram.offset + side * 36 + d * 3,
                    ap=[[0, C], [9, 4], [1, 3]],
                ),
            )
    # l96: cols 0..71 as above; cols 72..95 zero (gives the pts matmul an
    # all-zero 4th lhsT column so it can emit M=4 with row 3 = 0)
    l96 = setup.tile([C, 96], F32, tag="l96")
    nc.vector.memset(l96[:, 72:96], 0.0)
    oh_b72 = bass.AP(
        tensor=oh21.tensor, offset=oh21.offset,
        ap=[oh21.ap[0], [0, 6], [1, 4], [0, 3]],
    )
    nc.vector.tensor_tensor(out=l96[:, 0:72], in0=rt72, in1=oh_b72, op=OP.mult)
    # fold the -2 of the d2 cross term into the pred-side transform
    for d in range(3):
        nc.vector.tensor_scalar_mul(
            l96[:, d * 24 : d * 24 + 12], l96[:, d * 24 : d * 24 + 12], -2.0
        )

    # ---------------- sym flags ----------------
    sym_i = setup.tile([21, 1], I32, tag="sym_i")
    nc.sync.dma_start(out=sym_i, in_=h["sym"].ap())
    sym_f = setup.tile([21, 1], F32, tag="sym_f")
    nc.vector.tensor_copy(out=sym_f, in_=sym_i)
    ps_sym = psS.tile([1, 4], F32, tag="small")
    nc.tensor.matmul(ps_sym, lhsT=sym_f, rhs=oh21, start=True, stop=True)
    sym_row = acc.tile([1, 4], F32, tag="sym_row")
    nc.vector.tensor_copy(out=sym_row, in_=ps_sym)

    # ---------------- confidence loss: sum softplus(-x) ----------------
    e_scr = setup.tile([BPC, NCONF], F32, tag="e_scr")
    nc.scalar.activation(out=e_scr, in_=conf, func=AF.Exp, scale=-1.0)
    ln_scr = setup.tile([BPC, NCONF], F32, tag="ln_scr")
    sp_acc = setup.tile([BPC, 1], F32, tag="sp_acc")
    nc.scalar.activation(
        out=ln_scr, in_=e_scr, func=AF.Ln, bias=1.0, accum_out=sp_acc
    )
    ps_sp = psS.tile([1, 1], F32, tag="small")
    nc.tensor.matmul(ps_sp, lhsT=sp_acc, rhs=ones[0:BPC, :], start=True, stop=True)
    sp_sum = acc.tile([1, 1], F32, tag="sp_sum")
    nc.vector.tensor_copy(out=sp_sum, in_=ps_sp)

    # ---------------- pose regularization ----------------
    tsq = setup.tile([3, 4], F32, tag="tsq")
    nc.scalar.activation(out=tsq, in_=t4[0:3, 0:4], func=AF.Square)
    ps_tn = psS.tile([1, 4], F32, tag="small")
    nc.tensor.matmul(ps_tn, lhsT=ones[0:3, :], rhs=tsq, start=True, stop=True)
    tn = setup.tile([1, 4], F32, tag="tn")
    nc.scalar.activation(out=tn, in_=ps_tn, func=AF.Sqrt)
    bias_m2 = setup.tile([1, 1], F32, tag="bias_m2")
    nc.vector.memset(bias_m2, -2.0)
    rr = setup.tile([1, 4], F32, tag="rr")
    nc.scalar.activation(out=rr, in_=tn, func=AF.Relu, bias=bias_m2)
    rsq = setup.tile([1, 4], F32, tag="rsq")
    pr_acc = acc.tile([1, 1], F32, tag="pr_acc")
    nc.scalar.activation(out=rsq, in_=rr, func=AF.Square, accum_out=pr_acc)

    # ---------------- accumulators for the main loop ----------------
    colmin = acc.tile([128, BPC * 32], F32, tag="colmin")  # (b, m, half)
    addacc = acc.tile([1, BPC * 4], F32, tag="addacc")  # (b, nchunk)
    pnt_all = acc.tile([128, BPC * 16], F32, tag="pnt_all")  # pn, v-major

    # ---------------- main per-batch loop ----------------
    for b in range(BPC):
        # a4 rows: [-2(p+t) x3, 1];  g4 rows: [(g+t) x3, gn]
        a4 = pool["ab"].tile([4, V], F32, tag="a4")
        g4 = pool["ab"].tile([4, V], F32, tag="g4")
        pn1 = work.tile([1, V], F32, tag="pn1")

        for side in range(2):  # 0 = pred, 1 = gt
            j = side * 4 + b
            dst = a4 if side == 0 else g4
            for n in range(4):
                nsl = slice(n * 512, (n + 1) * 512)
                p4 = psS.tile([4, 512], F32, tag="small")
                # pred-side L columns carry the -2 factor; accumulate over i.
                # lhsT column 3 is all-zero, so row 3 starts at 0.
                for i in range(3):
                    s0 = side * 12 + b * 3 + i
                    nc.tensor.matmul(
                        p4,
                        lhsT=l96[:, s0 : s0 + 73 : 24],
                        rhs=vview[:, nsl, i : i + 1],
                        start=(i == 0),
                        stop=(i == 2),
                    )
                # squared true point coords (for pn / gn)
                sqc = work.tile([3, 512], F32, tag="sqc")
                nc.scalar.activation(
                    out=sqc, in_=p4[0:3, :], func=AF.Square,
                    bias=t4[0:3, j : j + 1],
                    scale=-0.5 if side == 0 else 1.0,
                )
                if side == 0:
                    # row 3 <- 1 (rank-1 fill); pn to its own psum strip
                    nc.tensor.matmul(
                        p4, lhsT=e1x4, rhs=ones512, start=False, stop=True,
                        skip_group_check=True,
                    )
                    ps_n = psS.tile([1, 512], F32, tag="small")
                    nc.tensor.matmul(
                        ps_n, lhsT=ones[0:3, :], rhs=sqc, start=True, stop=True
                    )
                    nc.scalar.copy(out=pn1[:, nsl], in_=ps_n)
                else:
                    # row 3 <- gn = sum of squared gt coords
                    nc.tensor.matmul(
                        p4, lhsT=e3x3, rhs=sqc, start=False, stop=True,
                        skip_group_check=True,
                    )
                nc.scalar.activation(
                    out=dst[:, nsl], in_=p4, func=AF.Identity,
                    bias=(bias_a if side == 0 else bias_g)[:, j : j + 1],
                    scale=1.0,
                )

        # pn -> v-major [128, 16] via DRAM bounce, one column DMA per m-chunk
        pn_dram = pool["dram"].tile([1, V], F32, tag="pn_dram")
        nc.sync.dma_start(out=pn_dram, in_=pn1)
        for m in range(16):
            nc.sync.dma_start(
                out=pnt_all[:, b * 16 + m : b * 16 + m + 1],
                in_=bass.AP(
                    tensor=pn_dram.tensor,
                    offset=pn_dram.offset + m * 128,
                    ap=[[1, 128]],
                ),
            )

        # ---- ADD (corresponding-point distance), true-difference form ----
        half = work.tile([3, V], F32, tag="half")
        nc.gpsimd.tensor_scalar_mul(half, a4[0:3, :], -0.5)
        diff = work.tile([3, V], F32, tag="diff")
        nc.gpsimd.tensor_sub(diff, half, g4[0:3, :])
        dsq = work.tile([3, V], F32, tag="dsq")
        nc.gpsimd.tensor_mul(dsq, diff, diff)
        for n in range(4):
            nsl = slice(n * 512, (n + 1) * 512)
            ps_da = psS.tile([1, 512], F32, tag="small")
            nc.tensor.matmul(
                ps_da, lhsT=ones[0:3, :], rhs=dsq[:, nsl], start=True, stop=True
            )
            da_scr = work.tile([1, 512], F32, tag="da_scr")
            nc.scalar.activation(
                out=da_scr, in_=ps_da, func=AF.Sqrt,
                accum_out=addacc[:, b * 4 + n : b * 4 + n + 1],
            )

        # ---- ADD-S: pairwise (gn - 2 p.g) matmuls + column-min reduce ----
        for m in range(16):
            msl = slice(m * 128, (m + 1) * 128)
            for nh in range(2):
                d2 = psB.tile([128, 1024], F32, tag="d2")
                for ns in range(2):
                    off = nh * 1024 + ns * 512
                    nc.tensor.matmul(
                        d2[:, ns * 512 : (ns + 1) * 512],
                        lhsT=a4[:, msl],
                        rhs=g4[:, off : off + 512],
                        start=True,
                        stop=True,
                    )
                col = (b * 16 + m) * 2 + nh
                # colmin[:, col] = min_w(gn - 2 p.g)  (pn added later)
                nc.vector.tensor_reduce(
                    out=colmin[:, col : col + 1], in_=d2, axis=AX.X, op=OP.min
                )

    # ---------------- epilogue ----------------
    mins2 = work.tile([128, BPC * 16], F32, tag="mins2")
    nc.vector.tensor_reduce(
        out=mins2, in_=colmin[:].rearrange("p (c h) -> p c h", h=2),
        axis=AX.X, op=OP.min,
    )
    minsp = work.tile([128, BPC * 16], F32, tag="minsp")
    nc.vector.tensor_add(minsp, mins2, pnt_all)
    minsc = work.tile([128, BPC * 16], F32, tag="minsc")
    nc.vector.tensor_scalar_max(minsc, minsp, 1e-12)
    sqm = work.tile([128, BPC * 16], F32, tag="sqm")
    nc.scalar.activation(out=sqm, in_=minsc, func=AF.Sqrt)
    ps_adds = psS.tile([1, BPC * 16], F32, tag="small")
    nc.tensor.matmul(ps_adds, lhsT=ones, rhs=sqm, start=True, stop=True)
    adds_s = work.tile([1, BPC], F32, tag="adds_s")
    nc.vector.tensor_reduce(
        out=adds_s, in_=ps_adds[:].rearrange("p (b m) -> p b m", b=BPC),
        axis=AX.X, op=OP.add,
    )
    adds_a = work.tile([1, BPC], F32, tag="adds_a")
    nc.vector.tensor_reduce(
        out=adds_a, in_=addacc[:].rearrange("p (b n) -> p b n", b=BPC),
        axis=AX.X, op=OP.add,
    )
    # sel = adds_a + sym * (adds_s - adds_a)
    dlt = work.tile([1, BPC], F32, tag="dlt")
    nc.vector.tensor_sub(dlt, adds_s, adds_a)
    dls = work.tile([1, BPC], F32, tag="dls")
    nc.vector.tensor_mul(dls, dlt, sym_row)
    sel = work.tile([1, BPC], F32, tag="sel")
    nc.vector.tensor_add(sel, adds_a, dls)
    selsum = work.tile([1, 1], F32, tag="selsum")
    nc.vector.tensor_reduce(out=selsum, in_=sel, axis=AX.X, op=OP.add)

    out_sb = acc.tile([1, 4], F32, tag="out_sb")
    nc.vector.tensor_copy(out=out_sb[:, 0:1], in_=selsum)
    nc.vector.tensor_copy(out=out_sb[:, 1:2], in_=sp_sum)
    nc.vector.tensor_copy(out=out_sb[:, 2:3], in_=pr_acc)
    nc.vector.memset(out_sb[:, 3:4], 0.0)
    nc.sync.dma_start(out=h["out"].ap(), in_=out_sb[:])


def build_nc():
    nc = bacc.Bacc("TRN2", target_bir_lowering=False, debug=False)
    h = {}
    h["poses"] = nc.dram_tensor("poses", [8, 7], F32, kind="ExternalInput")
    h["conf"] = nc.dram_tensor("conf", [BPC, NCONF], F32, kind="ExternalInput")
    h["cls"] = nc.dram_tensor("cls", [BPC], I32, kind="ExternalInput")
    h["verts"] = nc.dram_tensor("verts", [C, V, 3], F32, kind="ExternalInput")
    h["sym"] = nc.dram_tensor("sym", [C], I32, kind="ExternalInput")
    h["out"] = nc.dram_tensor("partial", [1, 4], F32, kind="ExternalOutput")
    h["iota21"] = nc.inline_tensor(
        np.arange(C, dtype=np.float32).reshape(21, 1), "iota21"
    )
    h["ones"] = nc.inline_tensor(np.ones((128, 1), np.float32), "ones128")
    h["ones512"] = nc.inline_tensor(np.ones((1, 512), np.float32), "ones512")
    h["mask_a"] = nc.inline_tensor(
        np.array([[-2.0], [-2.0], [-2.0], [0.0]], np.float32), "mask_a"
    )
    h["mask_g"] = nc.inline_tensor(
        np.array([[1.0], [1.0], [1.0], [0.0]], np.float32), "mask_g"
    )
    e3 = np.zeros((3, 4), np.float32)
    e3[:, 3] = 1.0
    h["e3x3"] = nc.inline_tensor(e3, "e3x3")
    e1 = np.zeros((1, 4), np.float32)
    e1[0, 3] = 1.0
    h["e1x4"] = nc.inline_tensor(e1, "e1x4")

    with tile.TileContext(nc) as tc, ExitStack() as ctx:
        _emit(nc, tc, h, ctx)
    nc.compile()
    return nc


def make_in_maps(pred_poses, gt_poses, pred_confidences, model_vertices, class_ids, sym_mask):
    pred_poses = np.asarray(pred_poses, np.float32)
    gt_poses = np.asarray(gt_poses, np.float32)
    pred_confidences = np.asarray(pred_confidences, np.float32)
    model_vertices = np.ascontiguousarray(np.asarray(model_vertices, np.float32))
    class_ids = np.asarray(class_ids, np.int32)
    sym_mask = np.asarray(sym_mask, np.int32)
    in_maps = []
    for i in range(NCORES):
        s = slice(i * BPC, (i + 1) * BPC)
        in_maps.append(
            {
                "poses": np.ascontiguousarray(
                    np.concatenate([pred_poses[s], gt_poses[s]], axis=0)
                ),
                "conf": np.ascontiguousarray(pred_confidences[s]),
                "cls": np.ascontiguousarray(class_ids[s]),
                "verts": model_vertices,
                "sym": sym_mask,
            }
        )
    return in_maps


def combine_partials(partials):
    partials = np.asarray(partials, np.float64)
    add_total = partials[:, 0].sum() / (B * V)
    conf_total = partials[:, 1].sum() / (B * NCONF)
    reg_total = partials[:, 2].sum() / B
    total = ADD_WEIGHT * add_total + CONF_WEIGHT * conf_total + POSE_REG_WEIGHT * reg_total
    return np.array(total, dtype=np.float32)


def kernel(**inputs):
    if "nc" not in _CACHE:
        _CACHE["nc"] = build_nc()
    nc = _CACHE["nc"]
    in_maps = make_in_maps(**inputs)
    res = run_bass_kernel_spmd(nc, in_maps, list(range(NCORES)))
    partials = np.stack([res.results[i]["partial"][0] for i in range(NCORES)])
    return combine_partials(partials)


# revision 37
# speedup vs baseline: 2.9572x; 2.9572x over previous
"""Trainium2 Bass kernel for nn_DenseFusionLoss (DenseFusion pose-estimation loss).

Strategy: data-parallel over the batch axis. Each of the 8 NeuronCores gets 4
batches (poses/confidences/class_ids shard) plus the full replicated
[21,2048,3] vertex table. Each core computes partial sums
[sum_selected_add_loss, sum_softplus, sum_pose_reg]; the host combines the 8
partial vectors into the final scalar loss.

Device-side computation per core (all heavy math on-device):
  - quat -> rotation matrices via unnormalized-product form (scaled by 1/|q|^2)
  - vertex gather fused into the point-transform matmul: lhsT[(c,i), d] =
    onehot[c] * R[d,i] (K=63), rhs = vertex table laid out [63, 2048]
  - pairwise squared distances d2[v,w] = pn[v] + gn[w] - 2 p.g via a K=5
    matmul with lhsT rows [-2px,-2py,-2pz, pn, 1], rhs rows [gx,gy,gz, 1, gn]
  - ADD-S: DVE reduce_min over PSUM d2 tiles, clamp, sqrt, mean
  - ADD: true-difference form on GPSIMD + PE partition-sum + ACT sqrt-accum
  - conf loss: softplus(-x) = Ln(1 + Exp(-x)) with ACT accumulate
  - pose reg: relu(|t|-2)^2 via ACT
"""

from contextlib import ExitStack

import numpy as np

import concourse.bass as bass
import concourse.bacc as bacc
import concourse.tile as tile
from concourse import mybir
from concourse.bass_utils import run_bass_kernel_spmd

B, C, V, NCONF = 32, 21, 2048, 1024
NCORES = 8
BPC = B // NCORES  # batches per core
F32 = mybir.dt.float32
F16 = mybir.dt.float16
I32 = mybir.dt.int32
AF = mybir.ActivationFunctionType
OP = mybir.AluOpType
AX = mybir.AxisListType

ADD_WEIGHT = 1.0
CONF_WEIGHT = 0.1
POSE_REG_WEIGHT = 0.1

_CACHE = {}


def _emit(nc, tc, h, ctx):
    pool = {}
    pool["setup"] = ctx.enter_context(tc.tile_pool(name="setup", bufs=1))
    pool["acc"] = ctx.enter_context(tc.tile_pool(name="acc", bufs=1))
    pool["ab"] = ctx.enter_context(tc.tile_pool(name="ab", bufs=2))
    pool["work"] = ctx.enter_context(tc.tile_pool(name="work", bufs=2))
    pool["psB"] = ctx.enter_context(tc.tile_pool(name="psB", bufs=3, space="PSUM"))
    pool["psS"] = ctx.enter_context(tc.tile_pool(name="psS", bufs=2, space="PSUM"))
    pool["dram"] = ctx.enter_context(tc.tile_pool(name="dram", bufs=1, space="DRAM"))

    setup = pool["setup"]
    acc = pool["acc"]
    work = pool["work"]
    psS = pool["psS"]
    psB = pool["psB"]

    # ---------------- constant / input loads ----------------
    iota21 = setup.tile([21, 1], F32, tag="iota21")
    nc.sync.dma_start(out=iota21, in_=h["iota21"].ap())
    ones = setup.tile([128, 1], F32, tag="ones")
    nc.sync.dma_start(out=ones, in_=h["ones"].ap())

    # vertex table, natural contiguous layout [c, (v i)], cast to fp16
    # during the (SWDGE) load; the pts matmul reads strided [21, 512]
    # coordinate-plane views out of it
    vnat = setup.tile([C, V * 3], F16, tag="vnat")
    nc.gpsimd.dma_start(out=vnat, in_=h["verts"].ap())
    vview = vnat[:].rearrange("c (v i) -> c v i", i=3)

    poses = setup.tile([8, 7], F32, tag="poses")
    nc.sync.dma_start(out=poses, in_=h["poses"].ap())

    # t4[d, j] = poses[j, d] for d in 0..3 (row 3 is junk, masked to 0 below)
    t4 = setup.tile([4, 8], F32, tag="t4")
    for j in range(8):
        nc.sync.dma_start(
            out=t4[:, j : j + 1],
            in_=bass.AP(tensor=h["poses"].ap().tensor, offset=j * 7, ap=[[1, 4]]),
        )
    mask_a = setup.tile([4, 1], F32, tag="mask_a")
    nc.sync.dma_start(out=mask_a, in_=h["mask_a"].ap())
    mask_g = setup.tile([4, 1], F32, tag="mask_g")
    nc.sync.dma_start(out=mask_g, in_=h["mask_g"].ap())
    e3x3 = setup.tile([3, 4], F16, tag="e3x3")
    nc.sync.dma_start(out=e3x3, in_=h["e3x3"].ap())
    e1x4 = setup.tile([1, 4], F16, tag="e1x4")
    nc.sync.dma_start(out=e1x4, in_=h["e1x4"].ap())
    ones512 = setup.tile([1, 512], F16, tag="ones512")
    nc.sync.dma_start(out=ones512, in_=h["ones512"].ap())
    ones3h = setup.tile([3, 1], F16, tag="ones3h")
    nc.sync.dma_start(out=ones3h, in_=h["ones3h"].ap())
    # per-pose ACT bias columns: rows 0-2 = (-2|1) * t, row 3 = 0
    bias_a = setup.tile([4, 8], F32, tag="bias_a")
    nc.vector.tensor_scalar(
        out=bias_a, in0=t4, scalar1=mask_a, scalar2=None, op0=OP.mult
    )
    bias_g = setup.tile([4, 8], F32, tag="bias_g")
    nc.vector.tensor_scalar(
        out=bias_g, in0=t4, scalar1=mask_g, scalar2=None, op0=OP.mult
    )

    conf = setup.tile([BPC, NCONF], F32, tag="conf")
    nc.sync.dma_start(out=conf, in_=h["conf"].ap())

    # ---------------- quaternion -> rotation matrices ----------------
    q = poses[:, 3:7]
    qsq = setup.tile([8, 4], F32, tag="qsq")
    nc.vector.tensor_mul(qsq, q, q)
    nrm2 = setup.tile([8, 1], F32, tag="nrm2")
    nc.vector.tensor_reduce(out=nrm2, in_=qsq, axis=AX.X, op=OP.add)
    inv2 = setup.tile([8, 1], F32, tag="inv2")
    nc.vector.reciprocal(inv2, nrm2)
    s2 = setup.tile([8, 1], F32, tag="s2")
    nc.vector.tensor_scalar_mul(s2, inv2, 2.0)
    ns2 = setup.tile([8, 1], F32, tag="ns2")
    nc.vector.tensor_scalar_mul(ns2, inv2, -2.0)

    # cross products: xy xz yz wx wy wz
    pr = setup.tile([8, 6], F32, tag="pr")
    nc.vector.tensor_mul(pr[:, 0:1], q[:, 1:2], q[:, 2:3])  # xy
    nc.vector.tensor_mul(pr[:, 1:2], q[:, 1:2], q[:, 3:4])  # xz
    nc.vector.tensor_mul(pr[:, 2:3], q[:, 2:3], q[:, 3:4])  # yz
    nc.vector.tensor_mul(pr[:, 3:4], q[:, 0:1], q[:, 1:2])  # wx
    nc.vector.tensor_mul(pr[:, 4:5], q[:, 0:1], q[:, 2:3])  # wy
    nc.vector.tensor_mul(pr[:, 5:6], q[:, 0:1], q[:, 3:4])  # wz

    xx, yy, zz = qsq[:, 1:2], qsq[:, 2:3], qsq[:, 3:4]
    xy, xz, yz = pr[:, 0:1], pr[:, 1:2], pr[:, 2:3]
    wx, wy, wz = pr[:, 3:4], pr[:, 4:5], pr[:, 5:6]

    sm = setup.tile([8, 9], F32, tag="sm")
    # entry e: (a op b); diag entries get ns2*sum + 1, off-diag s2*sum
    entries = [
        (yy, zz, OP.add, True),   # R00 = 1 - 2(yy+zz)/n2
        (xy, wz, OP.subtract, False),  # R01
        (xz, wy, OP.add, False),  # R02
        (xy, wz, OP.add, False),  # R10
        (xx, zz, OP.add, True),   # R11
        (yz, wx, OP.subtract, False),  # R12
        (xz, wy, OP.subtract, False),  # R20
        (yz, wx, OP.add, False),  # R21
        (xx, yy, OP.add, True),   # R22
    ]
    r_all = setup.tile([8, 9], F32, tag="r_all")
    for e, (a, b_, op, diag) in enumerate(entries):
        nc.vector.tensor_tensor(out=sm[:, e : e + 1], in0=a, in1=b_, op=op)
        nc.vector.tensor_scalar(
            out=r_all[:, e : e + 1],
            in0=sm[:, e : e + 1],
            scalar1=ns2 if diag else s2,
            scalar2=1.0 if diag else 0.0,
            op0=OP.mult,
            op1=OP.add,
        )

    # ---------------- one-hot class rows ----------------
    cls21 = setup.tile([21, 4], I32, tag="cls21")
    nc.gpsimd.dma_start(
        out=cls21,
        in_=bass.AP(tensor=h["cls"].ap().tensor, offset=0, ap=[[0, 21], [1, 4]]),
    )
    cls21f = setup.tile([21, 4], F32, tag="cls21f")
    nc.vector.tensor_copy(out=cls21f, in_=cls21)
    oh21 = setup.tile([21, 4], F32, tag="oh21")
    nc.vector.tensor_scalar(
        out=oh21, in0=cls21f, scalar1=iota21, scalar2=None, op0=OP.is_equal
    )

    # ---------------- replicated transform lhsT ----------------
    # l72[c, col], col = d*24 + side*12 + b*3 + i  holds
    # onehot_b(c) * R_side,b[d, i]  (pred side scaled by -2)
    r_dram = pool["dram"].tile([8, 9], F32, tag="r_dram")
    nc.sync.dma_start(out=r_dram, in_=r_all)
    rt72 = setup.tile([C, 72], F32, tag="rt72")
    for d in range(3):
        for side in range(2):
            # dst cols (b, i) at fixed (d, side); src r_dram[j, d*3+i]
            nc.gpsimd.dma_start(
                out=bass.AP(
                    tensor=rt72.tensor,
                    offset=rt72.offset + d * 24 + side * 12,
                    ap=[rt72.ap[0], [3, 4], [1, 3]],
                ),
                in_=bass.AP(
                    tensor=r_dram.tensor,
                    offset=r_dram.offset + side * 36 + d * 3,
                    ap=[[0, C], [9, 4], [1, 3]],
                ),
            )
    # l96: cols 0..71 as above; cols 72..95 zero (gives the pts matmul an
    # all-zero 4th lhsT column so it can emit M=4 with row 3 = 0)
    l96 = setup.tile([C, 96], F32, tag="l96")
    nc.vector.memset(l96[:, 72:96], 0.0)
    oh_b72 = bass.AP(
        tensor=oh21.tensor, offset=oh21.offset,
        ap=[oh21.ap[0], [0, 6], [1, 4], [0, 3]],
    )
    nc.vector.tensor_tensor(out=l96[:, 0:72], in0=rt72, in1=oh_b72, op=OP.mult)
    # fold the -2 of the d2 cross term into the pred-side transform
    for d in range(3):
        nc.vector.tensor_scalar_mul(
            l96[:, d * 24 : d * 24 + 12], l96[:, d * 24 : d * 24 + 12], -2.0
        )
    l96h = setup.tile([C, 96], F16, tag="l96h")
    nc.vector.tensor_copy(out=l96h, in_=l96)

    # ---------------- sym flags ----------------
    sym_i = setup.tile([21, 1], I32, tag="sym_i")
    nc.sync.dma_start(out=sym_i, in_=h["sym"].ap())
    sym_f = setup.tile([21, 1], F32, tag="sym_f")
    nc.vector.tensor_copy(out=sym_f, in_=sym_i)
    ps_sym = psS.tile([1, 4], F32, tag="small")
    nc.tensor.matmul(ps_sym, lhsT=sym_f, rhs=oh21, start=True, stop=True)
    sym_row = acc.tile([1, 4], F32, tag="sym_row")
    nc.vector.tensor_copy(out=sym_row, in_=ps_sym)

    # ---------------- confidence loss: sum softplus(-x) ----------------
    e_scr = setup.tile([BPC, NCONF], F32, tag="e_scr")
    nc.scalar.activation(out=e_scr, in_=conf, func=AF.Exp, scale=-1.0)
    ln_scr = setup.tile([BPC, NCONF], F32, tag="ln_scr")
    sp_acc = setup.tile([BPC, 1], F32, tag="sp_acc")
    nc.scalar.activation(
        out=ln_scr, in_=e_scr, func=AF.Ln, bias=1.0, accum_out=sp_acc
    )
    ps_sp = psS.tile([1, 1], F32, tag="small")
    nc.tensor.matmul(ps_sp, lhsT=sp_acc, rhs=ones[0:BPC, :], start=True, stop=True)
    sp_sum = acc.tile([1, 1], F32, tag="sp_sum")
    nc.vector.tensor_copy(out=sp_sum, in_=ps_sp)

    # ---------------- pose regularization ----------------
    tsq = setup.tile([3, 4], F32, tag="tsq")
    nc.scalar.activation(out=tsq, in_=t4[0:3, 0:4], func=AF.Square)
    ps_tn = psS.tile([1, 4], F32, tag="small")
    nc.tensor.matmul(ps_tn, lhsT=ones[0:3, :], rhs=tsq, start=True, stop=True)
    tn = setup.tile([1, 4], F32, tag="tn")
    nc.scalar.activation(out=tn, in_=ps_tn, func=AF.Sqrt)
    bias_m2 = setup.tile([1, 1], F32, tag="bias_m2")
    nc.vector.memset(bias_m2, -2.0)
    rr = setup.tile([1, 4], F32, tag="rr")
    nc.scalar.activation(out=rr, in_=tn, func=AF.Relu, bias=bias_m2)
    rsq = setup.tile([1, 4], F32, tag="rsq")
    pr_acc = acc.tile([1, 1], F32, tag="pr_acc")
    nc.scalar.activation(out=rsq, in_=rr, func=AF.Square, accum_out=pr_acc)

    # ---------------- accumulators for the main loop ----------------
    colmin = acc.tile([128, BPC * 32], F32, tag="colmin")  # (b, m, half)
    addacc = acc.tile([1, BPC * 4], F32, tag="addacc")  # (b, nchunk)
    pnt_all = acc.tile([128, BPC * 16], F32, tag="pnt_all")  # pn, v-major

    # ---------------- main per-batch loop ----------------
    for b in range(BPC):
        # a4 rows: [-2(p+t) x3, 1];  g4 rows: [(g+t) x3, gn]   (both fp16)
        a4 = pool["ab"].tile([4, V], F16, tag="a4")
        g4 = pool["ab"].tile([4, V], F16, tag="g4")
        pn1 = work.tile([1, V], F32, tag="pn1")

        for side in range(2):  # 0 = pred, 1 = gt
            j = side * 4 + b
            dst = a4 if side == 0 else g4
            for n in range(4):
                nsl = slice(n * 512, (n + 1) * 512)
                p4 = psS.tile([4, 512], F32, tag="small")
                # pred-side L columns carry the -2 factor; accumulate over i.
                # lhsT column 3 is all-zero, so row 3 starts at 0.
                for i in range(3):
                    s0 = side * 12 + b * 3 + i
                    nc.tensor.matmul(
                        p4,
                        lhsT=l96h[:, s0 : s0 + 73 : 24],
                        rhs=vview[:, nsl, i : i + 1],
                        start=(i == 0),
                        stop=(i == 2),
                    )
                # squared true point coords (for pn / gn)
                sqc = work.tile([3, 512], F16, tag="sqc")
                nc.scalar.activation(
                    out=sqc, in_=p4[0:3, :], func=AF.Square,
                    bias=t4[0:3, j : j + 1],
                    scale=-0.5 if side == 0 else 1.0,
                )
                if side == 0:
                    # row 3 <- 1 (rank-1 fill); pn to its own psum strip
                    nc.tensor.matmul(
                        p4, lhsT=e1x4, rhs=ones512, start=False, stop=True,
                        skip_group_check=True,
                    )
                    ps_n = psS.tile([1, 512], F32, tag="small")
                    nc.tensor.matmul(
                        ps_n, lhsT=ones3h, rhs=sqc, start=True, stop=True
                    )
                    nc.scalar.copy(out=pn1[:, nsl], in_=ps_n)
                else:
                    # row 3 <- gn = sum of squared gt coords
                    nc.tensor.matmul(
                        p4, lhsT=e3x3, rhs=sqc, start=False, stop=True,
                        skip_group_check=True,
                    )
                nc.scalar.activation(
                    out=dst[:, nsl], in_=p4, func=AF.Identity,
                    bias=(bias_a if side == 0 else bias_g)[:, j : j + 1],
                    scale=1.0,
                )

        # pn -> v-major [128, 16] via DRAM bounce, one column DMA per m-chunk
        pn_dram = pool["dram"].tile([1, V], F32, tag="pn_dram")
        nc.sync.dma_start(out=pn_dram, in_=pn1)
        for m in range(16):
            nc.sync.dma_start(
                out=pnt_all[:, b * 16 + m : b * 16 + m + 1],
                in_=bass.AP(
                    tensor=pn_dram.tensor,
                    offset=pn_dram.offset + m * 128,
                    ap=[[1, 128]],
                ),
            )

        # ---- ADD (corresponding-point distance), true-difference form ----
        # u = 0.5*a4 + g4 = -(p+t_p) + (g+t_g) = -diff;  sum_d u^2 = |diff|^2
        h2 = work.tile([3, V], F32, tag="h2")
        nc.scalar.activation(out=h2, in_=a4[0:3, :], func=AF.Copy, scale=0.5)
        u = work.tile([3, V], F32, tag="u")
        nc.gpsimd.tensor_add(u, h2, g4[0:3, :])
        usq = work.tile([3, V], F16, tag="usq")
        nc.scalar.activation(out=usq, in_=u, func=AF.Square)
        for n in range(4):
            nsl = slice(n * 512, (n + 1) * 512)
            ps_da = psS.tile([1, 512], F32, tag="small")
            nc.tensor.matmul(
                ps_da, lhsT=ones3h, rhs=usq[:, nsl], start=True, stop=True
            )
            da_scr = work.tile([1, 512], F32, tag="da_scr")
            nc.scalar.activation(
                out=da_scr, in_=ps_da, func=AF.Sqrt,
                accum_out=addacc[:, b * 4 + n : b * 4 + n + 1],
            )

        # ---- ADD-S: pairwise (gn - 2 p.g) matmuls + column-min reduce ----
        for m in range(16):
            msl = slice(m * 128, (m + 1) * 128)
            for nh in range(2):
                d2 = psB.tile([128, 1024], F32, tag="d2")
                for ns in range(2):
                    off = nh * 1024 + ns * 512
                    nc.tensor.matmul(
                        d2[:, ns * 512 : (ns + 1) * 512],
                        lhsT=a4[:, msl],
                        rhs=g4[:, off : off + 512],
                        start=True,
                        stop=True,
                    )
                col = (b * 16 + m) * 2 + nh
                # colmin[:, col] = min_w(gn - 2 p.g)  (pn added later)
                nc.vector.tensor_reduce(
                    out=colmin[:, col : col + 1], in_=d2, axis=AX.X, op=OP.min
                )

    # ---------------- epilogue ----------------
    mins2 = work.tile([128, BPC * 16], F32, tag="mins2")
    nc.vector.tensor_reduce(
        out=mins2, in_=colmin[:].rearrange("p (c h) -> p c h", h=2),
        axis=AX.X, op=OP.min,
    )
    minsp = work.tile([128, BPC * 16], F32, tag="minsp")
    nc.vector.tensor_add(minsp, mins2, pnt_all)
    minsc = work.tile([128, BPC * 16], F32, tag="minsc")
    nc.vector.tensor_scalar_max(minsc, minsp, 1e-12)
    sqm = work.tile([128, BPC * 16], F32, tag="sqm")
    nc.scalar.activation(out=sqm, in_=minsc, func=AF.Sqrt)
    ps_adds = psS.tile([1, BPC * 16], F32, tag="small")
    nc.tensor.matmul(ps_adds, lhsT=ones, rhs=sqm, start=True, stop=True)
    adds_s = work.tile([1, BPC], F32, tag="adds_s")
    nc.vector.tensor_reduce(
        out=adds_s, in_=ps_adds[:].rearrange("p (b m) -> p b m", b=BPC),
        axis=AX.X, op=OP.add,
    )
    adds_a = work.tile([1, BPC], F32, tag="adds_a")
    nc.vector.tensor_reduce(
        out=adds_a, in_=addacc[:].rearrange("p (b n) -> p b n", b=BPC),
        axis=AX.X, op=OP.add,
    )
    # sel = adds_a + sym * (adds_s - adds_a)
    dlt = work.tile([1, BPC], F32, tag="dlt")
    nc.vector.tensor_sub(dlt, adds_s, adds_a)
    dls = work.tile([1, BPC], F32, tag="dls")
    nc.vector.tensor_mul(dls, dlt, sym_row)
    sel = work.tile([1, BPC], F32, tag="sel")
    nc.vector.tensor_add(sel, adds_a, dls)
    selsum = work.tile([1, 1], F32, tag="selsum")
    nc.vector.tensor_reduce(out=selsum, in_=sel, axis=AX.X, op=OP.add)

    out_sb = acc.tile([1, 4], F32, tag="out_sb")
    nc.vector.tensor_copy(out=out_sb[:, 0:1], in_=selsum)
    nc.vector.tensor_copy(out=out_sb[:, 1:2], in_=sp_sum)
    nc.vector.tensor_copy(out=out_sb[:, 2:3], in_=pr_acc)
    nc.vector.memset(out_sb[:, 3:4], 0.0)
    nc.sync.dma_start(out=h["out"].ap(), in_=out_sb[:])


def build_nc():
    nc = bacc.Bacc("TRN2", target_bir_lowering=False, debug=False)
    h = {}
    h["poses"] = nc.dram_tensor("poses", [8, 7], F32, kind="ExternalInput")
    h["conf"] = nc.dram_tensor("conf", [BPC, NCONF], F32, kind="ExternalInput")
    h["cls"] = nc.dram_tensor("cls", [BPC], I32, kind="ExternalInput")
    h["verts"] = nc.dram_tensor("verts", [C, V, 3], F32, kind="ExternalInput")
    h["sym"] = nc.dram_tensor("sym", [C], I32, kind="ExternalInput")
    h["out"] = nc.dram_tensor("partial", [1, 4], F32, kind="ExternalOutput")
    h["iota21"] = nc.inline_tensor(
        np.arange(C, dtype=np.float32).reshape(21, 1), "iota21"
    )
    h["ones"] = nc.inline_tensor(np.ones((128, 1), np.float32), "ones128")
    h["ones512"] = nc.inline_tensor(np.ones((1, 512), np.float16), "ones512")
    h["ones3h"] = nc.inline_tensor(np.ones((3, 1), np.float16), "ones3h")
    h["mask_a"] = nc.inline_tensor(
        np.array([[-2.0], [-2.0], [-2.0], [0.0]], np.float32), "mask_a"
    )
    h["mask_g"] = nc.inline_tensor(
        np.array([[1.0], [1.0], [1.0], [0.0]], np.float32), "mask_g"
    )
    e3 = np.zeros((3, 4), np.float16)
    e3[:, 3] = 1.0
    h["e3x3"] = nc.inline_tensor(e3, "e3x3")
    e1 = np.zeros((1, 4), np.float16)
    e1[0, 3] = 1.0
    h["e1x4"] = nc.inline_tensor(e1, "e1x4")

    with tile.TileContext(nc) as tc, ExitStack() as ctx:
        _emit(nc, tc, h, ctx)
    nc.compile()
    return nc


def make_in_maps(pred_poses, gt_poses, pred_confidences, model_vertices, class_ids, sym_mask):
    pred_poses = np.asarray(pred_poses, np.float32)
    gt_poses = np.asarray(gt_poses, np.float32)
    pred_confidences = np.asarray(pred_confidences, np.float32)
    model_vertices = np.ascontiguousarray(np.asarray(model_vertices, np.float32))
    class_ids = np.asarray(class_ids, np.int32)
    sym_mask = np.asarray(sym_mask, np.int32)
    in_maps = []
    for i in range(NCORES):
        s = slice(i * BPC, (i + 1) * BPC)
        in_maps.append(
            {
                "poses": np.ascontiguousarray(
                    np.concatenate([pred_poses[s], gt_poses[s]], axis=0)
                ),
                "conf": np.ascontiguousarray(pred_confidences[s]),
                "cls": np.ascontiguousarray(class_ids[s]),
                "verts": model_vertices,
                "sym": sym_mask,
            }
        )
    return in_maps


def combine_partials(partials):
    partials = np.asarray(partials, np.float64)
    add_total = partials[:, 0].sum() / (B * V)
    conf_total = partials[:, 1].sum() / (B * NCONF)
    reg_total = partials[:, 2].sum() / B
    total = ADD_WEIGHT * add_total + CONF_WEIGHT * conf_total + POSE_REG_WEIGHT * reg_total
    return np.array(total, dtype=np.float32)


def kernel(**inputs):
    if "nc" not in _CACHE:
        _CACHE["nc"] = build_nc()
    nc = _CACHE["nc"]
    in_maps = make_in_maps(**inputs)
    res = run_bass_kernel_spmd(nc, in_maps, list(range(NCORES)))
    partials = np.stack([res.results[i]["partial"][0] for i in range(NCORES)])
    return combine_partials(partials)
